# revision 41
# baseline (speedup 1.0000x reference)
"""Multi-head attention (B=4, S=2048, D=1024, H=16) on 8 Trainium2 cores.

Sharding: core c handles batch b = c//2 and head-group hg = c%2 (8 of the 16
heads, i.e. 512 of the 1024 projection dims).  Every core computes:

    Qc^T = (Wq_cols^T @ q[b]^T)           [512, 2048]   (proj-major layout)
    Kc^T = (Wk_cols^T @ k[b]^T)           [512, 2048]
    Vc   = (v[b] @ Wv_cols)               [2048, 512]
    S^T  = Kc_h @ Qc_h^T per head         (scores, transposed: [keys, queries])
    P^T  = exp(S^T/8 + maskbias)          (ACT engine, fused scale+mask)
    A^T  = V_h^T @ P^T   and  l = 1^T P^T (AV + denominator via matmul)
    A^T  = A^T * (1/l)                    (selector-matmul bcast + approx 1/x)
    out_partial = A_c @ Wo_rows           [2048, 1024]

Host sums the two partial outputs per batch (the "all-reduce after w_o")
and adds the folded bias bv @ Wo + bo.  Biases bq/bk are applied on-device
(per-partition adds); the mask is applied as an additive bias inside the
exp activation.

v3 schedule: one software-pipelined stream.  Queries are processed in
512-wide chunks (qq); for each (qq, head-pair) window the two heads' score
chunks for a key block land in ONE [128,1024] PSUM tile (they share the key
chunk, hence the mask bias), so each exp instruction covers 1024
elements/lane while the AV accumulators shrink to one PSUM bank per head.
The spare banks host projection/output-projection chunks, which are woven
into the attention windows as PE filler, so the scalar engine's exp stream
(the ~290us floor) starts ~25us in and the tensor engine (the ~330us floor)
rarely idles.  The softmax normalization runs inline at each window end
(selector-matmul broadcast of the denominator rows, reciprocal_approx_fast,
multiply).  NOTE: the V ones-columns (softmax denominator ride-along) are
initialized with a gpsimd memset, NOT an element-strided scatter DMA — that
scatter's HW-DGE completion accounting is unreliable (queue fan-out varies
by transfer shape) and intermittently let AV matmuls read unwritten ones,
corrupting results nondeterministically.
"""

import os
import numpy as np

B, S, D = 4, 2048, 1024
H, DK = 16, 64
P = 128
NCORES = 8
HPC = H // 2            # heads per core
PROJ = HPC * DK         # 512 projection dims per core
NDM = D // P            # 8 d_model chunks
NPC = PROJ // P         # 4 head-pair chunks
NSC = S // 512          # 4 seq chunks of 512
NSO = S // P            # 16 seq chunks of 128
NKC = S // P            # 16 key chunks of 128
NQQ = S // 512          # 4 query chunks of 512

MASK_NEG = -30000.0     # exp(x - 30000) == 0 in fp32 for any plausible x

_cache = {}


def _build():
    """Build + compile the per-core Bass program (same program on all cores)."""
    import concourse.bass as bass
    import concourse.bacc as bacc
    import concourse.mybir as mybir
    import concourse.tile as tile
    from contextlib import ExitStack
    import functools

    f32 = mybir.dt.float32
    bf16 = mybir.dt.bfloat16
    AF = mybir.ActivationFunctionType
    MUL = mybir.AluOpType.mult

    nc = bacc.Bacc("TRN2", target_bir_lowering=False, debug=False,
                   num_devices=NCORES)

    # activations host-preblocked to [P, NSC, NDM, 512]: partition p, seq
    # chunk sc reads one contiguous 8KB run; weights likewise.
    qTb = nc.dram_tensor("qTb", [P, NSC, NDM, 512], bf16, kind="ExternalInput").ap()
    kTb = nc.dram_tensor("kTb", [P, NSC, NDM, 512], bf16, kind="ExternalInput").ap()
    vTb = nc.dram_tensor("vTb", [P, NSC, NDM, 512], bf16, kind="ExternalInput").ap()
    wq = nc.dram_tensor("wq", [P, NDM, PROJ], bf16, kind="ExternalInput").ap()
    wk = nc.dram_tensor("wk", [P, NDM, PROJ], bf16, kind="ExternalInput").ap()
    wv = nc.dram_tensor("wv", [P, NDM, PROJ], bf16, kind="ExternalInput").ap()
    wo = nc.dram_tensor("wo", [P, NPC, D], bf16, kind="ExternalInput").ap()
    bq2 = nc.dram_tensor("bq2", [P, NPC], f32, kind="ExternalInput").ap()
    bk2 = nc.dram_tensor("bk2", [P, NPC], f32, kind="ExternalInput").ap()
    mb = nc.dram_tensor("mb", [P, NKC], f32, kind="ExternalInput").ap()
    sel = nc.dram_tensor("sel", [P, 128], f32, kind="ExternalInput").ap()
    out = nc.dram_tensor("out", [S, D], f32, kind="ExternalOutput").ap()

    with tile.TileContext(nc) as tc, ExitStack() as ctx:
        # ---------------- persistent SBUF ----------------
        respool = ctx.enter_context(tc.tile_pool(name="res", bufs=1))
        QT_sb = respool.tile([P, NPC, S], bf16)   # [pair-chunk, seq]
        # K^T per head on the full 128-partition contraction range: even heads
        # in rows 0-63, odd heads in rows 64-127, zeros elsewhere, so the S^T
        # matmul is a standard 128-contraction matmul against the pair-stacked
        # Q^T (no PE tiling modes: those throttle the PE clock gate).
        KT_sb = respool.tile([P, HPC, S], bf16)
        nc.vector.memset(KT_sb[:], 0.0)
        # V with an interleaved ones column per head: head h occupies
        # cols [h*65, h*65+64) and col h*65+64 == 1.0 (softmax denominator
        # rides along the AV matmul as output partition 64).
        V_sb = respool.tile([P, NSO, HPC * (DK + 1)], bf16)
        # ones-columns via memset (the projection copies overwrite the data
        # columns): an element-strided vones scatter-DMA here is exactly the
        # queue-fanout shape whose completion accounting is unreliable.
        nc.gpsimd.memset(V_sb[:], 1.0)
        AT_sb = respool.tile([P, NPC, S], bf16)   # normalized A^T

        cpool = ctx.enter_context(tc.tile_pool(name="const", bufs=1))
        sel_sb = cpool.tile([P, 128], f32)
        mb_sb = cpool.tile([P, NKC], f32)
        bq_sb = cpool.tile([P, NPC], f32)
        bk_sb = cpool.tile([P, NPC], f32)
        # persistent denominator staging: rows 0 and 32 are overwritten each
        # window; the rest stays 1.0 so the approx reciprocal (undefined on
        # zero) sees benign values in the unused rows.
        Lsb = cpool.tile([P, 512], f32)
        nc.gpsimd.memset(Lsb[:], 1.0)

        # ---------------- staging pools ----------------
        wpool = ctx.enter_context(tc.tile_pool(name="w", bufs=1))
        akpool = ctx.enter_context(tc.tile_pool(name="ak", bufs=4))
        aqpool = ctx.enter_context(tc.tile_pool(name="aq", bufs=2))
        avpool = ctx.enter_context(tc.tile_pool(name="av", bufs=2))
        wopool = ctx.enter_context(tc.tile_pool(name="wo", bufs=1))
        epool = ctx.enter_context(tc.tile_pool(name="expS", bufs=4))
        rpool = ctx.enter_context(tc.tile_pool(name="rcp", bufs=2))
        opool = ctx.enter_context(tc.tile_pool(name="ostage", bufs=3))

        # ---------------- PSUM pools (8 banks total) ----------------
        # psS : scores, [128,1024] = 2 banks x bufs=2          -> 4 banks
        # psAcc: AV accumulators, 2 tags x [65,512] = 1 bank   -> 2 banks
        # psX : projection / output-proj chunks [128,512] x 2  -> 2 banks
        psS = ctx.enter_context(tc.tile_pool(name="psS", bufs=2, space="PSUM"))
        psAcc = ctx.enter_context(tc.tile_pool(name="psAcc", bufs=1, space="PSUM"))
        psX = ctx.enter_context(tc.tile_pool(name="psX", bufs=2, space="PSUM"))

        # ---------------- work-unit emitters ----------------
        w_tiles = {}

        def load_w(name, w_hbm):
            w_sb = wpool.tile([P, NDM, PROJ], bf16, tag=f"w{name}",
                              name=f"w_{name}")
            nc.sync.dma_start(w_sb[:], w_hbm)
            w_tiles[name] = w_sb

        a_tiles = {}

        def load_act(name, x_hbm, sc):
            pool = {"k": akpool, "q": aqpool, "v": avpool}[name]
            a_sb = pool.tile([P, NDM, 512], bf16, tag=f"a{name}",
                             name=f"a_{name}{sc}")
            nc.sync.dma_start(a_sb[:], x_hbm[:, sc])
            a_tiles[(name, sc)] = a_sb

        def proj_qk(name, bias_sb, dst, pc, sc):
            """One [128proj x 512seq] chunk of the Q^T / K^T projection."""
            w_sb = w_tiles[name]
            a_sb = a_tiles[(name, sc)]
            ps = psX.tile([P, 512], f32, tag="px", name="px")
            for dc in range(NDM):
                nc.tensor.matmul(
                    ps,
                    lhsT=w_sb[:, dc, pc * P:(pc + 1) * P],
                    rhs=a_sb[:, dc, :],
                    start=(dc == 0), stop=(dc == NDM - 1),
                )
            if dst is QT_sb:
                nc.vector.tensor_scalar_add(
                    dst[:, pc, sc * 512:(sc + 1) * 512], ps,
                    bias_sb[:, pc:pc + 1])
            else:
                for half in range(2):
                    lo = half * 64
                    nc.vector.tensor_scalar_add(
                        KT_sb[lo:lo + 64, 2 * pc + half,
                              sc * 512:(sc + 1) * 512],
                        ps[lo:lo + 64, :],
                        bias_sb[lo:lo + 64, pc:pc + 1])

        def proj_v(sc, so4):
            """One [128seq x 512proj] chunk of the V projection."""
            w_sb = w_tiles["v"]
            a_sb = a_tiles[("v", sc)]
            so = sc * 4 + so4
            ps = psX.tile([P, 512], f32, tag="px", name="px")
            for dc in range(NDM):
                nc.tensor.matmul(
                    ps,
                    lhsT=a_sb[:, dc, so4 * P:(so4 + 1) * P],
                    rhs=w_sb[:, dc, :],
                    start=(dc == 0), stop=(dc == NDM - 1),
                )
            nc.vector.tensor_copy(
                V_sb[:, so, :].rearrange(
                    "p (h w) -> p h w", w=DK + 1)[:, :, 0:DK],
                ps.rearrange("p (h w) -> p h w", w=DK))

        wo_sb = wopool.tile([P, NPC, D], bf16)

        def out_chunk(so, oc):
            """One [128seq x 512dmodel] chunk of the output projection."""
            ps = psX.tile([P, 512], f32, tag="px", name="px")
            for pc in range(NPC):
                nc.tensor.matmul(
                    ps,
                    lhsT=AT_sb[:, pc, so * P:(so + 1) * P],
                    rhs=wo_sb[:, pc, oc * 512:(oc + 1) * 512],
                    start=(pc == 0), stop=(pc == NPC - 1),
                )
            ost = opool.tile([P, 512], f32, tag="o", name="ost")
            nc.vector.tensor_copy(ost, ps)
            nc.sync.dma_start(
                out[so * P:(so + 1) * P, oc * 512:(oc + 1) * 512], ost)

        # ---------------- attention window B(pr, qq) ----------------
        # Per key-chunk kc: two score matmuls (one per head, N=512) into ONE
        # [128,1024] PSUM tile -> one exp (1024/lane) -> two AV matmuls
        # accumulating into [65,512] per head.  Scores run 2 kc ahead of AV.
        # At window end the AV accumulators (+ denominator rows) are staged
        # to SBUF so the PSUM banks free immediately; the returned closure
        # finishes the normalization and is run as the next window's first
        # filler.
        def attention(pr, qq, fillers):
            qlo = qq * 512
            es = {}

            def scores(kc):
                sp = psS.tile([P, 1024], f32, tag="s", name="sp")
                for hi in range(2):
                    nc.tensor.matmul(
                        sp[:, hi * 512:(hi + 1) * 512],
                        lhsT=KT_sb[:, 2 * pr + hi, kc * P:(kc + 1) * P],
                        rhs=QT_sb[:, pr, qlo:qlo + 512],
                        start=True, stop=True,
                    )
                e = epool.tile([P, 1024], bf16, tag="e", name="e")
                nc.scalar.activation(
                    e, sp, AF.Exp,
                    bias=mb_sb[:, kc:kc + 1],
                    scale=float(1.0 / np.sqrt(DK)),
                )
                es[kc] = e

            scores(0)
            scores(1)

            avs = [psAcc.tile([P, 512], f32, tag=f"av{hi}", name=f"av{hi}")
                   for hi in range(2)]

            def av(kc):
                e = es.pop(kc)
                for hi in range(2):
                    h = 2 * pr + hi
                    nc.tensor.matmul(
                        avs[hi][0:DK + 1, :],
                        lhsT=V_sb[:, kc, h * (DK + 1):(h + 1) * (DK + 1)],
                        rhs=e[:, hi * 512:(hi + 1) * 512],
                        start=(kc == 0), stop=(kc == NKC - 1),
                    )

            nfill = len(fillers)
            fdone = 0
            for kc in range(NKC):
                if kc + 2 < NKC:
                    scores(kc + 2)
                # drain fillers front-loaded (1/kc) so cross-window deps and
                # the V chunks consumed by this window's own AVs land in time
                want = min(nfill, kc + 1)
                while fdone < want:
                    fillers[fdone]()
                    fdone += 1
                av(kc)
            while fdone < nfill:
                fillers[fdone]()
                fdone += 1

            # normalization, inline at window end (v2-proven lifetimes):
            # pull the denominator rows, broadcast via the selector matmul,
            # approx-reciprocal, multiply the PSUM accumulators into AT_sb.
            for hi in range(2):
                nc.vector.tensor_copy(
                    Lsb[hi * 32:hi * 32 + 1, :], avs[hi][DK:DK + 1, :])
            bc = psX.tile([P, 512], f32, tag="px", name="bc")
            nc.tensor.matmul(bc, lhsT=sel_sb[:], rhs=Lsb[:],
                             start=True, stop=True)
            rc = rpool.tile([P, 512], f32, tag="rc", name="rc")
            nc.vector.reciprocal_approx_fast(out=rc, in_=bc)
            for hi in range(2):
                nc.vector.tensor_tensor(
                    AT_sb[hi * 64:(hi + 1) * 64, pr, qlo:qlo + 512],
                    avs[hi][0:64, :],
                    rc[hi * 64:(hi + 1) * 64, :], MUL)

        # ---------------- emission schedule ----------------
        # DMA order: consts, K-path (wk + all kT), Q/V-path chunk 0, rest.
        wk_sb = wpool.tile([P, NDM, PROJ], bf16, tag="wk", name="w_k")
        nc.sync.dma_start(wk_sb[:, :, 0:P], wk[:, :, 0:P])
        nc.sync.dma_start(wk_sb[:, :, P:PROJ], wk[:, :, P:PROJ])
        w_tiles["k"] = wk_sb
        k0_sb = akpool.tile([P, NDM, 512], bf16, tag="ak", name="a_k0")
        nc.sync.dma_start(k0_sb[:, 0:4], kTb[:, 0, 0:4])
        nc.sync.dma_start(k0_sb[:, 4:8], kTb[:, 0, 4:8])
        a_tiles[("k", 0)] = k0_sb
        nc.sync.dma_start(bk_sb[:], bk2)
        nc.sync.dma_start(bq_sb[:], bq2)
        nc.sync.dma_start(mb_sb[:], mb)
        nc.sync.dma_start(sel_sb[:], sel)
        for sc in range(1, NSC):
            load_act("k", kTb, sc)
        load_w("q", wq)
        load_act("q", qTb, 0)
        load_w("v", wv)
        load_act("v", vTb, 0)
        nc.sync.dma_start(wo_sb[:], wo)
        for sc in range(1, NSC):
            load_act("q", qTb, sc)
            load_act("v", vTb, sc)

        # Prelude: K(pc0) + Q(pc0,sc0) + V(sc0) so the exp stream can start
        # immediately; everything else becomes filler inside the windows.
        for sc in range(NSC):
            proj_qk("k", bk_sb, KT_sb, 0, sc)
        proj_qk("q", bq_sb, QT_sb, 0, 0)
        for so4 in range(4):
            proj_v(0, so4)

        # filler inventory.  Window sequence is (qq0: pr0..3), (qq1: pr0..3),
        # ...  Fillers assigned to window W are emitted DURING W, so they may
        # only carry deps of LATER windows (or, for (0,0), the V chunks its
        # own later-kc AVs need — drained front-loaded, 1 per kc).
        filler_q = []
        win = {}  # (pr,qq) -> list of emitters

        def add(prqq, fn):
            win.setdefault(prqq, []).append(fn)

        # B(0,0): V(sc1..3) early (kc 4/8/12 need them), then K(pc1)+Q(pc1,0)
        # for window (1,0).
        for sc in range(1, NSC):
            for so4 in range(4):
                add((0, 0), functools.partial(proj_v, sc, so4))
        for sc in range(NSC):
            add((0, 0), functools.partial(proj_qk, "k", bk_sb, KT_sb, 1, sc))
        add((0, 0), functools.partial(proj_qk, "q", bq_sb, QT_sb, 1, 0))
        # B(1,0): K(pc2)+Q(pc2,0) for window (2,0)
        for sc in range(NSC):
            add((1, 0), functools.partial(proj_qk, "k", bk_sb, KT_sb, 2, sc))
        add((1, 0), functools.partial(proj_qk, "q", bq_sb, QT_sb, 2, 0))
        # B(2,0): K(pc3)+Q(pc3,0) for window (3,0)
        for sc in range(NSC):
            add((2, 0), functools.partial(proj_qk, "k", bk_sb, KT_sb, 3, sc))
        add((2, 0), functools.partial(proj_qk, "q", bq_sb, QT_sb, 3, 0))
        # B(3,0): Q(pc0,1) for window (0,1)
        add((3, 0), functools.partial(proj_qk, "q", bq_sb, QT_sb, 0, 1))
        # remaining Q chunks: window (pr, qq) carries Q for window (pr+1, qq),
        # wrapping to (0, qq+1)
        for qq in range(1, NQQ):
            add((0, qq), functools.partial(proj_qk, "q", bq_sb, QT_sb, 1, qq))
            add((1, qq), functools.partial(proj_qk, "q", bq_sb, QT_sb, 2, qq))
            add((2, qq), functools.partial(proj_qk, "q", bq_sb, QT_sb, 3, qq))
            if qq + 1 < NQQ:
                add((3, qq),
                    functools.partial(proj_qk, "q", bq_sb, QT_sb, 0, qq + 1))
        # output-projection chunks for qq spread across the qq+1 windows
        for qq in range(NQQ):
            tgt = [(pr2, qq + 1) for pr2 in range(4)] if qq + 1 < NQQ else None
            ci = 0
            for so in range(qq * 4, qq * 4 + 4):
                for oc in range(2):
                    if tgt is None:
                        filler_q.append(functools.partial(out_chunk, so, oc))
                    else:
                        add(tgt[ci % 4], functools.partial(out_chunk, so, oc))
                        ci += 1

        # run the windows
        for qq in range(NQQ):
            for pr in range(4):
                attention(pr, qq, win.get((pr, qq), []))
        # tail: last query-chunk's output projection
        for fn in filler_q:
            fn()

    nc.compile()
    return nc


def _get_nc():
    if "nc" not in _cache:
        _cache["nc"] = _build()
    return _cache["nc"]


def make_in_maps(q, k, v, mask, Wq, bq, Wk, bk, Wv, bv, Wo, bo):
    """Host-side sharding: slice/transpose/block the full inputs per core."""
    import ml_dtypes
    f = np.float32
    bf = ml_dtypes.bfloat16
    q = np.asarray(q, dtype=f)
    k = np.asarray(k, dtype=f)
    v = np.asarray(v, dtype=f)
    Wq = np.asarray(Wq, dtype=f)
    Wk = np.asarray(Wk, dtype=f)
    Wv = np.asarray(Wv, dtype=f)
    Wo = np.asarray(Wo, dtype=f)
    bq = np.asarray(bq, dtype=f)
    bk = np.asarray(bk, dtype=f)
    mask = np.asarray(mask)

    def block_act(xT):
        # [D, S] -> [P, NSC, NDM, 512]: partition p, seq-chunk sc contiguous
        return np.ascontiguousarray(
            xT.reshape(NDM, P, NSC, 512).transpose(1, 2, 0, 3)).astype(bf)

    def block_w(W):
        # [D, PROJ] -> [P, NDM, PROJ]
        return np.ascontiguousarray(
            W.reshape(NDM, P, PROJ).transpose(1, 0, 2)).astype(bf)

    sel_np = np.zeros((P, 128), dtype=f)
    sel_np[0, 0:64] = 1.0
    sel_np[32, 64:128] = 1.0

    in_maps = []
    for c in range(NCORES):
        b, hg = divmod(c, 2)
        cols = slice(hg * PROJ, (hg + 1) * PROJ)
        mbias = np.where(mask[b, 0, 0, :] == 0, f(MASK_NEG), f(0.0)).astype(f)
        in_maps.append({
            "qTb": block_act(q[b].T),
            "kTb": block_act(k[b].T),
            "vTb": block_act(v[b].T),
            "wq": block_w(Wq[:, cols]),
            "wk": block_w(Wk[:, cols]),
            "wv": block_w(Wv[:, cols]),
            "wo": np.ascontiguousarray(
                Wo[cols, :].reshape(NPC, P, D).transpose(1, 0, 2)).astype(bf),
            "bq2": np.ascontiguousarray(bq[cols].reshape(NPC, P).T),
            "bk2": np.ascontiguousarray(bk[cols].reshape(NPC, P).T),
            "mb": np.ascontiguousarray(mbias.reshape(NKC, P).T),
            "sel": sel_np,
        })
    return in_maps


def combine_outputs(parts, Wv_bv_Wo_bo):
    """Host-side unshard: sum the two head-group partials per batch, add the
    folded bias bv @ Wo + bo."""
    bv, Wo, bo = Wv_bv_Wo_bo
    bo_eff = (np.asarray(bv, np.float32) @ np.asarray(Wo, np.float32)
              + np.asarray(bo, np.float32))
    out = np.empty((B, S, D), dtype=np.float32)
    for b in range(B):
        out[b] = parts[2 * b] + parts[2 * b + 1] + bo_eff
    return out


def _install_axon_ntff_hook():
    """The agent image's antenv lacks axon_hooks; synthesize it and register
    the ctypes NTFF profile hook from trn_boot so trace=True works."""
    import sys
    import types
    if "antenv.axon_hooks" in sys.modules:
        return
    try:
        from trn_agent_boot.trn_boot import _ntff_profile_via_ctypes
        hook = _ntff_profile_via_ctypes("/opt/axon/libaxon_pjrt.so")
    except Exception:
        hook = None
    mod = types.ModuleType("antenv.axon_hooks")
    mod._hook = hook
    mod.get_axon_ntff_profile_hook = lambda: mod._hook
    mod.set_axon_ntff_profile_hook = lambda h: setattr(mod, "_hook", h)
    sys.modules["antenv.axon_hooks"] = mod
    # upload_artifacts wants a fish bucket; keep artifacts local instead.
    import concourse.bass_utils as bu
    bu.upload_artifacts = lambda tmpdir: str(tmpdir)


def kernel(q, k, v, mask, Wq, bq, Wk, bk, Wv, bv, Wo, bo):
    from concourse.bass_utils import run_bass_kernel_spmd

    nc = _get_nc()
    in_maps = make_in_maps(q, k, v, mask, Wq, bq, Wk, bk, Wv, bv, Wo, bo)
    trace = bool(int(os.environ.get("KERNEL_TRACE", "0")))
    if trace:
        try:
            _install_axon_ntff_hook()
        except Exception:
            trace = False
    try:
        res = run_bass_kernel_spmd(
            nc, in_maps, list(range(NCORES)), trace=trace,
            tmpdir=os.environ.get("KERNEL_TRACE_DIR") or None)
    except Exception:
        if not trace:
            raise
        # Trace machinery failed; rerun without it so results still flow.
        res = run_bass_kernel_spmd(nc, in_maps, list(range(NCORES)), trace=False)
    _cache["last_result"] = res
    parts = [res.results[c]["out"] for c in range(NCORES)]
    return combine_outputs(parts, (bv, Wo, bo))


# revision 44
# speedup vs baseline: 1.1773x; 1.1773x over previous
"""Multi-head attention (B=4, S=2048, D=1024, H=16) on 8 Trainium2 cores.

Sharding: core c handles batch b = c//2 and head-group hg = c%2 (8 of the 16
heads, i.e. 512 of the 1024 projection dims).  Every core computes:

    Qc^T = (Wq_cols^T @ q[b]^T)           [512, 2048]   (proj-major layout)
    Kc^T = (Wk_cols^T @ k[b]^T)           [512, 2048]
    Vc   = (v[b] @ Wv_cols)               [2048, 512]
    S^T  = Kc_h @ Qc_h^T per head         (scores, transposed: [keys, queries])
    P^T  = exp(S^T/8 + maskbias)          (ACT engine, fused scale+mask)
    A^T  = V_h^T @ P^T   and  l = 1^T P^T (AV + denominator via matmul)
    A^T  = A^T * (1/l)                    (selector-matmul bcast + approx 1/x)
    out_partial = A_c @ Wo_rows           [2048, 1024]

Host sums the two partial outputs per batch (the "all-reduce after w_o")
and adds the folded bias bv @ Wo + bo.  Biases bq/bk are applied on-device
(per-partition adds); the mask is applied as an additive bias inside the
exp activation.

v3 schedule: one software-pipelined stream.  Queries are processed in
512-wide chunks (qq); for each (qq, head-pair) window the two heads' score
chunks for a key block land in ONE [128,1024] PSUM tile (they share the key
chunk, hence the mask bias), so each exp instruction covers 1024
elements/lane while the AV accumulators shrink to one PSUM bank per head.
The spare banks host projection/output-projection chunks, which are woven
into the attention windows as PE filler, so the scalar engine's exp stream
(the ~290us floor) starts ~25us in and the tensor engine (the ~330us floor)
rarely idles.  The softmax normalization runs inline at each window end
(selector-matmul broadcast of the denominator rows, reciprocal_approx_fast,
multiply).  NOTE: the V ones-columns (softmax denominator ride-along) are
initialized with a gpsimd memset, NOT an element-strided scatter DMA — that
scatter's HW-DGE completion accounting is unreliable (queue fan-out varies
by transfer shape) and intermittently let AV matmuls read unwritten ones,
corrupting results nondeterministically.
"""

import os
import numpy as np

B, S, D = 4, 2048, 1024
H, DK = 16, 64
P = 128
NCORES = 8
HPC = H // 2            # heads per core
PROJ = HPC * DK         # 512 projection dims per core
NDM = D // P            # 8 d_model chunks
NPC = PROJ // P         # 4 head-pair chunks
NSC = S // 512          # 4 seq chunks of 512
NSO = S // P            # 16 seq chunks of 128
NKC = S // P            # 16 key chunks of 128
NQQ = S // 512          # 4 query chunks of 512

MASK_NEG = -30000.0     # exp(x - 30000) == 0 in fp32 for any plausible x

_cache = {}


def _build():
    """Build + compile the per-core Bass program (same program on all cores)."""
    import concourse.bass as bass
    import concourse.bacc as bacc
    import concourse.mybir as mybir
    import concourse.tile as tile
    from contextlib import ExitStack
    import functools

    f32 = mybir.dt.float32
    bf16 = mybir.dt.bfloat16
    AF = mybir.ActivationFunctionType
    MUL = mybir.AluOpType.mult

    nc = bacc.Bacc("TRN2", target_bir_lowering=False, debug=False,
                   num_devices=NCORES)

    # activations host-preblocked to [P, NSC, NDM, 512]: partition p, seq
    # chunk sc reads one contiguous 8KB run; weights likewise.
    qTb = nc.dram_tensor("qTb", [P, NSC, NDM, 512], bf16, kind="ExternalInput").ap()
    kTb = nc.dram_tensor("kTb", [P, NSC, NDM, 512], bf16, kind="ExternalInput").ap()
    vTb = nc.dram_tensor("vTb", [P, NSC, NDM, 512], bf16, kind="ExternalInput").ap()
    wq = nc.dram_tensor("wq", [P, NDM, PROJ], bf16, kind="ExternalInput").ap()
    wk = nc.dram_tensor("wk", [P, NDM, PROJ], bf16, kind="ExternalInput").ap()
    wv = nc.dram_tensor("wv", [P, NDM, PROJ], bf16, kind="ExternalInput").ap()
    wo = nc.dram_tensor("wo", [P, NPC, D], bf16, kind="ExternalInput").ap()
    bq2 = nc.dram_tensor("bq2", [P, NPC], f32, kind="ExternalInput").ap()
    bk2 = nc.dram_tensor("bk2", [P, NPC], f32, kind="ExternalInput").ap()
    mb = nc.dram_tensor("mb", [P, NKC], f32, kind="ExternalInput").ap()
    sel = nc.dram_tensor("sel", [P, 128], f32, kind="ExternalInput").ap()
    out = nc.dram_tensor("out", [S, D], f32, kind="ExternalOutput").ap()

    with tile.TileContext(nc) as tc, ExitStack() as ctx:
        # ---------------- persistent SBUF ----------------
        respool = ctx.enter_context(tc.tile_pool(name="res", bufs=1))
        QT_sb = respool.tile([P, NPC, S], bf16)   # [pair-chunk, seq]
        # K^T per head on the full 128-partition contraction range: even heads
        # in rows 0-63, odd heads in rows 64-127, zeros elsewhere, so the S^T
        # matmul is a standard 128-contraction matmul against the pair-stacked
        # Q^T (no PE tiling modes: those throttle the PE clock gate).
        KT_sb = respool.tile([P, HPC, S], bf16)
        nc.vector.memset(KT_sb[:], 0.0)
        # V with an interleaved ones column per head: head h occupies
        # cols [h*65, h*65+64) and col h*65+64 == 1.0 (softmax denominator
        # rides along the AV matmul as output partition 64).
        V_sb = respool.tile([P, NSO, HPC * (DK + 1)], bf16)
        # ones-columns via memset (the projection copies overwrite the data
        # columns): an element-strided vones scatter-DMA here is exactly the
        # queue-fanout shape whose completion accounting is unreliable.
        nc.gpsimd.memset(V_sb[:], 1.0)
        AT_sb = respool.tile([P, NPC, S], bf16)   # normalized A^T

        cpool = ctx.enter_context(tc.tile_pool(name="const", bufs=1))
        sel_sb = cpool.tile([P, 128], f32)
        nc.sync.dma_start(sel_sb[:], sel)
        mb_sb = cpool.tile([P, NKC], f32)
        nc.sync.dma_start(mb_sb[:], mb)
        bq_sb = cpool.tile([P, NPC], f32)
        nc.sync.dma_start(bq_sb[:], bq2)
        bk_sb = cpool.tile([P, NPC], f32)
        nc.sync.dma_start(bk_sb[:], bk2)
        # persistent denominator staging: rows 0 and 32 are overwritten each
        # window; the rest stays 1.0 so the approx reciprocal (undefined on
        # zero) sees benign values in the unused rows.
        Lsb = cpool.tile([P, 512], f32)
        nc.gpsimd.memset(Lsb[:], 1.0)

        # ---------------- staging pools ----------------
        wpool = ctx.enter_context(tc.tile_pool(name="w", bufs=1))
        akpool = ctx.enter_context(tc.tile_pool(name="ak", bufs=4))
        aqpool = ctx.enter_context(tc.tile_pool(name="aq", bufs=2))
        avpool = ctx.enter_context(tc.tile_pool(name="av", bufs=2))
        wopool = ctx.enter_context(tc.tile_pool(name="wo", bufs=1))
        epool = ctx.enter_context(tc.tile_pool(name="expS", bufs=4))
        rpool = ctx.enter_context(tc.tile_pool(name="rcp", bufs=2))
        opool = ctx.enter_context(tc.tile_pool(name="ostage", bufs=3))

        # ---------------- PSUM pools (8 banks total) ----------------
        # psS : scores, [128,1024] = 2 banks x bufs=2          -> 4 banks
        # psAcc: AV accumulators, 2 tags x [65,512] = 1 bank   -> 2 banks
        # psX : projection / output-proj chunks [128,512] x 2  -> 2 banks
        psS = ctx.enter_context(tc.tile_pool(name="psS", bufs=2, space="PSUM"))
        psAcc = ctx.enter_context(tc.tile_pool(name="psAcc", bufs=1, space="PSUM"))
        psX = ctx.enter_context(tc.tile_pool(name="psX", bufs=2, space="PSUM"))

        # ---------------- work-unit emitters ----------------
        w_tiles = {}

        def load_w(name, w_hbm):
            w_sb = wpool.tile([P, NDM, PROJ], bf16, tag=f"w{name}",
                              name=f"w_{name}")
            nc.sync.dma_start(w_sb[:], w_hbm)
            w_tiles[name] = w_sb

        a_tiles = {}

        def load_act(name, x_hbm, sc):
            pool = {"k": akpool, "q": aqpool, "v": avpool}[name]
            a_sb = pool.tile([P, NDM, 512], bf16, tag=f"a{name}",
                             name=f"a_{name}{sc}")
            nc.sync.dma_start(a_sb[:], x_hbm[:, sc])
            a_tiles[(name, sc)] = a_sb

        def proj_qk(name, bias_sb, dst, pc, sc):
            """One [128proj x 512seq] chunk of the Q^T / K^T projection."""
            w_sb = w_tiles[name]
            a_sb = a_tiles[(name, sc)]
            ps = psX.tile([P, 512], f32, tag="px", name="px")
            for dc in range(NDM):
                nc.tensor.matmul(
                    ps,
                    lhsT=w_sb[:, dc, pc * P:(pc + 1) * P],
                    rhs=a_sb[:, dc, :],
                    start=(dc == 0), stop=(dc == NDM - 1),
                )
            if dst is QT_sb:
                nc.vector.tensor_scalar_add(
                    dst[:, pc, sc * 512:(sc + 1) * 512], ps,
                    bias_sb[:, pc:pc + 1])
            else:
                for half in range(2):
                    lo = half * 64
                    nc.vector.tensor_scalar_add(
                        KT_sb[lo:lo + 64, 2 * pc + half,
                              sc * 512:(sc + 1) * 512],
                        ps[lo:lo + 64, :],
                        bias_sb[lo:lo + 64, pc:pc + 1])

        def proj_v(sc, so4):
            """One [128seq x 512proj] chunk of the V projection."""
            w_sb = w_tiles["v"]
            a_sb = a_tiles[("v", sc)]
            so = sc * 4 + so4
            ps = psX.tile([P, 512], f32, tag="px", name="px")
            for dc in range(NDM):
                nc.tensor.matmul(
                    ps,
                    lhsT=a_sb[:, dc, so4 * P:(so4 + 1) * P],
                    rhs=w_sb[:, dc, :],
                    start=(dc == 0), stop=(dc == NDM - 1),
                )
            nc.vector.tensor_copy(
                V_sb[:, so, :].rearrange(
                    "p (h w) -> p h w", w=DK + 1)[:, :, 0:DK],
                ps.rearrange("p (h w) -> p h w", w=DK))

        wo_sb = wopool.tile([P, NPC, D], bf16)

        def out_chunk(so, oc):
            """One [128seq x 512dmodel] chunk of the output projection."""
            ps = psX.tile([P, 512], f32, tag="px", name="px")
            for pc in range(NPC):
                nc.tensor.matmul(
                    ps,
                    lhsT=AT_sb[:, pc, so * P:(so + 1) * P],
                    rhs=wo_sb[:, pc, oc * 512:(oc + 1) * 512],
                    start=(pc == 0), stop=(pc == NPC - 1),
                )
            ost = opool.tile([P, 512], f32, tag="o", name="ost")
            nc.vector.tensor_copy(ost, ps)
            nc.sync.dma_start(
                out[so * P:(so + 1) * P, oc * 512:(oc + 1) * 512], ost)

        # ---------------- attention window B(pr, qq) ----------------
        # Per key-chunk kc: two score matmuls (one per head, N=512) into ONE
        # [128,1024] PSUM tile -> one exp (1024/lane) -> two AV matmuls
        # accumulating into [65,512] per head.  Scores run 2 kc ahead of AV.
        # At window end the AV accumulators (+ denominator rows) are staged
        # to SBUF so the PSUM banks free immediately; the returned closure
        # finishes the normalization and is run as the next window's first
        # filler.
        def attention(pr, qq, fillers):
            qlo = qq * 512
            es = {}

            def scores(kc):
                sp = psS.tile([P, 1024], f32, tag="s", name="sp")
                for hi in range(2):
                    nc.tensor.matmul(
                        sp[:, hi * 512:(hi + 1) * 512],
                        lhsT=KT_sb[:, 2 * pr + hi, kc * P:(kc + 1) * P],
                        rhs=QT_sb[:, pr, qlo:qlo + 512],
                        start=True, stop=True,
                    )
                e = epool.tile([P, 1024], bf16, tag="e", name="e")
                nc.scalar.activation(
                    e, sp, AF.Exp,
                    bias=mb_sb[:, kc:kc + 1],
                    scale=float(1.0 / np.sqrt(DK)),
                )
                es[kc] = e

            scores(0)
            scores(1)

            avs = [psAcc.tile([P, 512], f32, tag=f"av{hi}", name=f"av{hi}")
                   for hi in range(2)]

            def av(kc):
                e = es.pop(kc)
                for hi in range(2):
                    h = 2 * pr + hi
                    nc.tensor.matmul(
                        avs[hi][0:DK + 1, :],
                        lhsT=V_sb[:, kc, h * (DK + 1):(h + 1) * (DK + 1)],
                        rhs=e[:, hi * 512:(hi + 1) * 512],
                        start=(kc == 0), stop=(kc == NKC - 1),
                    )

            nfill = len(fillers)
            fdone = 0
            for kc in range(NKC):
                if kc + 2 < NKC:
                    scores(kc + 2)
                # drain fillers front-loaded (1/kc) so cross-window deps and
                # the V chunks consumed by this window's own AVs land in time
                want = min(nfill, kc + 1)
                while fdone < want:
                    fillers[fdone]()
                    fdone += 1
                av(kc)
            while fdone < nfill:
                fillers[fdone]()
                fdone += 1

            # normalization, inline at window end (v2-proven lifetimes):
            # pull the denominator rows, broadcast via the selector matmul,
            # approx-reciprocal, multiply the PSUM accumulators into AT_sb.
            for hi in range(2):
                nc.vector.tensor_copy(
                    Lsb[hi * 32:hi * 32 + 1, :], avs[hi][DK:DK + 1, :])
            bc = psS.tile([P, 512], f32, tag="s", name="bc")
            nc.tensor.matmul(bc, lhsT=sel_sb[:], rhs=Lsb[:],
                             start=True, stop=True)
            rc = rpool.tile([P, 512], f32, tag="rc", name="rc")
            nc.vector.reciprocal_approx_fast(out=rc, in_=bc)
            for hi in range(2):
                nc.vector.tensor_tensor(
                    AT_sb[hi * 64:(hi + 1) * 64, pr, qlo:qlo + 512],
                    avs[hi][0:64, :],
                    rc[hi * 64:(hi + 1) * 64, :], MUL)

        # ---------------- emission schedule ----------------
        # DMA order: consts, K-path (wk + all kT), Q/V-path chunk 0, rest.
        wk_sb = wpool.tile([P, NDM, PROJ], bf16, tag="wk", name="w_k")
        nc.sync.dma_start(wk_sb[:, :, 0:P], wk[:, :, 0:P])
        nc.sync.dma_start(wk_sb[:, :, P:PROJ], wk[:, :, P:PROJ])
        w_tiles["k"] = wk_sb
        k0_sb = akpool.tile([P, NDM, 512], bf16, tag="ak", name="a_k0")
        nc.sync.dma_start(k0_sb[:, 0:4], kTb[:, 0, 0:4])
        nc.sync.dma_start(k0_sb[:, 4:8], kTb[:, 0, 4:8])
        a_tiles[("k", 0)] = k0_sb
        for sc in range(1, NSC):
            load_act("k", kTb, sc)
        load_w("q", wq)
        load_act("q", qTb, 0)
        load_w("v", wv)
        load_act("v", vTb, 0)
        nc.sync.dma_start(wo_sb[:], wo)
        for sc in range(1, NSC):
            load_act("q", qTb, sc)
            load_act("v", vTb, sc)

        # Prelude: K(pc0) + Q(pc0,sc0) + V(sc0) so the exp stream can start
        # immediately; everything else becomes filler inside the windows.
        for sc in range(NSC):
            proj_qk("k", bk_sb, KT_sb, 0, sc)
        proj_qk("q", bq_sb, QT_sb, 0, 0)
        for so4 in range(4):
            proj_v(0, so4)

        # filler inventory.  Window sequence is (qq0: pr0..3), (qq1: pr0..3),
        # ...  Fillers assigned to window W are emitted DURING W, so they may
        # only carry deps of LATER windows (or, for (0,0), the V chunks its
        # own later-kc AVs need — drained front-loaded, 1 per kc).
        filler_q = []
        win = {}  # (pr,qq) -> list of emitters

        def add(prqq, fn):
            win.setdefault(prqq, []).append(fn)

        # B(0,0): V(sc1..3) early (kc 4/8/12 need them), then K(pc1)+Q(pc1,0)
        # for window (1,0).
        for sc in range(1, NSC):
            for so4 in range(4):
                add((0, 0), functools.partial(proj_v, sc, so4))
        for sc in range(NSC):
            add((0, 0), functools.partial(proj_qk, "k", bk_sb, KT_sb, 1, sc))
        add((0, 0), functools.partial(proj_qk, "q", bq_sb, QT_sb, 1, 0))
        # B(1,0): K(pc2)+Q(pc2,0) for window (2,0)
        for sc in range(NSC):
            add((1, 0), functools.partial(proj_qk, "k", bk_sb, KT_sb, 2, sc))
        add((1, 0), functools.partial(proj_qk, "q", bq_sb, QT_sb, 2, 0))
        # B(2,0): K(pc3)+Q(pc3,0) for window (3,0)
        for sc in range(NSC):
            add((2, 0), functools.partial(proj_qk, "k", bk_sb, KT_sb, 3, sc))
        add((2, 0), functools.partial(proj_qk, "q", bq_sb, QT_sb, 3, 0))
        # B(3,0): Q(pc0,1) for window (0,1)
        add((3, 0), functools.partial(proj_qk, "q", bq_sb, QT_sb, 0, 1))
        # remaining Q chunks: window (pr, qq) carries Q for window (pr+1, qq),
        # wrapping to (0, qq+1)
        for qq in range(1, NQQ):
            add((0, qq), functools.partial(proj_qk, "q", bq_sb, QT_sb, 1, qq))
            add((1, qq), functools.partial(proj_qk, "q", bq_sb, QT_sb, 2, qq))
            add((2, qq), functools.partial(proj_qk, "q", bq_sb, QT_sb, 3, qq))
            if qq + 1 < NQQ:
                add((3, qq),
                    functools.partial(proj_qk, "q", bq_sb, QT_sb, 0, qq + 1))
        # output-projection chunks for qq spread across the qq+1 windows
        for qq in range(NQQ):
            tgt = [(pr2, qq + 1) for pr2 in range(4)] if qq + 1 < NQQ else None
            ci = 0
            for so in range(qq * 4, qq * 4 + 4):
                for oc in range(2):
                    if tgt is None:
                        filler_q.append(functools.partial(out_chunk, so, oc))
                    else:
                        add(tgt[ci % 4], functools.partial(out_chunk, so, oc))
                        ci += 1

        # run the windows
        for qq in range(NQQ):
            for pr in range(4):
                attention(pr, qq, win.get((pr, qq), []))
        # tail: last query-chunk's output projection
        for fn in filler_q:
            fn()

    nc.compile()
    return nc


def _get_nc():
    if "nc" not in _cache:
        _cache["nc"] = _build()
    return _cache["nc"]


def make_in_maps(q, k, v, mask, Wq, bq, Wk, bk, Wv, bv, Wo, bo):
    """Host-side sharding: slice/transpose/block the full inputs per core."""
    import ml_dtypes
    f = np.float32
    bf = ml_dtypes.bfloat16
    q = np.asarray(q, dtype=f)
    k = np.asarray(k, dtype=f)
    v = np.asarray(v, dtype=f)
    Wq = np.asarray(Wq, dtype=f)
    Wk = np.asarray(Wk, dtype=f)
    Wv = np.asarray(Wv, dtype=f)
    Wo = np.asarray(Wo, dtype=f)
    bq = np.asarray(bq, dtype=f)
    bk = np.asarray(bk, dtype=f)
    mask = np.asarray(mask)

    def block_act(xT):
        # [D, S] -> [P, NSC, NDM, 512]: partition p, seq-chunk sc contiguous
        return np.ascontiguousarray(
            xT.reshape(NDM, P, NSC, 512).transpose(1, 2, 0, 3)).astype(bf)

    def block_w(W):
        # [D, PROJ] -> [P, NDM, PROJ]
        return np.ascontiguousarray(
            W.reshape(NDM, P, PROJ).transpose(1, 0, 2)).astype(bf)

    sel_np = np.zeros((P, 128), dtype=f)
    sel_np[0, 0:64] = 1.0
    sel_np[32, 64:128] = 1.0

    in_maps = []
    for c in range(NCORES):
        b, hg = divmod(c, 2)
        cols = slice(hg * PROJ, (hg + 1) * PROJ)
        mbias = np.where(mask[b, 0, 0, :] == 0, f(MASK_NEG), f(0.0)).astype(f)
        in_maps.append({
            "qTb": block_act(q[b].T),
            "kTb": block_act(k[b].T),
            "vTb": block_act(v[b].T),
            "wq": block_w(Wq[:, cols]),
            "wk": block_w(Wk[:, cols]),
            "wv": block_w(Wv[:, cols]),
            "wo": np.ascontiguousarray(
                Wo[cols, :].reshape(NPC, P, D).transpose(1, 0, 2)).astype(bf),
            "bq2": np.ascontiguousarray(bq[cols].reshape(NPC, P).T),
            "bk2": np.ascontiguousarray(bk[cols].reshape(NPC, P).T),
            "mb": np.ascontiguousarray(mbias.reshape(NKC, P).T),
            "sel": sel_np,
        })
    return in_maps


def combine_outputs(parts, Wv_bv_Wo_bo):
    """Host-side unshard: sum the two head-group partials per batch, add the
    folded bias bv @ Wo + bo."""
    bv, Wo, bo = Wv_bv_Wo_bo
    bo_eff = (np.asarray(bv, np.float32) @ np.asarray(Wo, np.float32)
              + np.asarray(bo, np.float32))
    out = np.empty((B, S, D), dtype=np.float32)
    for b in range(B):
        out[b] = parts[2 * b] + parts[2 * b + 1] + bo_eff
    return out


def _install_axon_ntff_hook():
    """The agent image's antenv lacks axon_hooks; synthesize it and register
    the ctypes NTFF profile hook from trn_boot so trace=True works."""
    import sys
    import types
    if "antenv.axon_hooks" in sys.modules:
        return
    try:
        from trn_agent_boot.trn_boot import _ntff_profile_via_ctypes
        hook = _ntff_profile_via_ctypes("/opt/axon/libaxon_pjrt.so")
    except Exception:
        hook = None
    mod = types.ModuleType("antenv.axon_hooks")
    mod._hook = hook
    mod.get_axon_ntff_profile_hook = lambda: mod._hook
    mod.set_axon_ntff_profile_hook = lambda h: setattr(mod, "_hook", h)
    sys.modules["antenv.axon_hooks"] = mod
    # upload_artifacts wants a fish bucket; keep artifacts local instead.
    import concourse.bass_utils as bu
    bu.upload_artifacts = lambda tmpdir: str(tmpdir)


def kernel(q, k, v, mask, Wq, bq, Wk, bk, Wv, bv, Wo, bo):
    from concourse.bass_utils import run_bass_kernel_spmd

    nc = _get_nc()
    in_maps = make_in_maps(q, k, v, mask, Wq, bq, Wk, bk, Wv, bv, Wo, bo)
    trace = bool(int(os.environ.get("KERNEL_TRACE", "0")))
    if trace:
        try:
            _install_axon_ntff_hook()
        except Exception:
            trace = False
    try:
        res = run_bass_kernel_spmd(
            nc, in_maps, list(range(NCORES)), trace=trace,
            tmpdir=os.environ.get("KERNEL_TRACE_DIR") or None)
    except Exception:
        if not trace:
            raise
        # Trace machinery failed; rerun without it so results still flow.
        res = run_bass_kernel_spmd(nc, in_maps, list(range(NCORES)), trace=False)
    _cache["last_result"] = res
    parts = [res.results[c]["out"] for c in range(NCORES)]
    return combine_outputs(parts, (bv, Wo, bo))
